# revision 18
# baseline (speedup 1.0000x reference)
"""Trainium2 Bass kernel for nn_GNNLayer (gnn_message_passing).

Math: out = (A1 @ xf.T).T @ W.T + b  with xf = x.reshape(B, -1).

Key structural facts (from the COO construction, deterministic):
  * every row/col index is < 4103 (the builder uses `k + j`, not
    `k*lng*lat + j`), so only a 4103-wide corner of the 32768-dim space
    participates;
  * the coalesced A1[:4103, :4103] is banded: col-row offsets all lie in
    [-72, 72], and its values are small integers (<= 12) — exactly
    representable in bf16.

So the computation reduces exactly to
  out = xf[:, :M] @ A1s.T @ W[:, :M].T + b ,  M = 4103,
with A1s banded.  We run it as dense 128x128 band-block matmuls on the
TensorEngine, sharding the M dimension across the 8 cores (5 m-tiles of
128 per core, zero-padded to 5120).  Each core produces a partial
(128, 256) output; the host sums the 8 partials and adds the bias.

Precision/speed scheme: x is split into bf16 hi + bf16 lo (together ~17
mantissa bits); the band matrix is exact in bf16.  The SpMM stage then
runs as bf16 matmuls (1 cycle/row on the PE instead of 4 for fp32) with
free dim 256 ([hi | lo] packed), accumulating in fp32 PSUM.  The
projection with W runs in fp32r (free dim 256 -> 1 cycle/row; HW-probed
matmul rel err ~1.5e-4, which dominates the end-to-end error and sits
far below the fp32 envelope of the scatter-add reference).  The hi+lo
sum folds into the projection as two lhsT slices accumulating into the
same PSUM bank, so each h1 tile needs only one DVE PSUM->SBUF copy.

Per core:
  h1ps[m, 0:128 | 128:256] = sum_j band_blk[t,j].T @ [x_hi | x_lo]_blk
  out[b, o] = sum_t sum_half h1[t][:, half].T @ WsT_tile[t]  (fp32r, PSUM)

Host-side work is data reformatting only (transposes / COO->dense band
scatter / bf16 split / final unshard-reduce); every FLOP involving x or
W runs on the NeuronCores.
"""

import ml_dtypes
import numpy as np

BF16 = ml_dtypes.bfloat16

B = 128          # batch
OUT = 256        # linear output dim
N = 32768        # full node count
M = 4103         # highest touched index + 1 (structural, verified at runtime)
HALF_BAND = 72   # |col - row| <= 72 for every COO entry
NCORES = 8
TPC = 5          # 128-wide m-tiles per core
CHUNK = 128 * TPC            # 640 m-indices per core
M_PAD = NCORES * CHUNK       # 5120
KSUB = TPC + 2               # 7 k-subtiles of xk per core
N_WARM = 5                   # PE warm-up matmuls (HAM ramp) during DMA phase

# xk hi/lo packed layout: block j occupies columns [256j, 256j+256) =
# [hi_j | lo_j].  Split point between the two xk DMA chunks (in blocks):
XK_SPLIT = 5                 # blocks 0-4 in chunk A, 5-6 in chunk B
BAND_SPLIT = 3               # band tiles 0-2 in chunk A, 3-4 in chunk B
WST_SPLIT = 3                # W tiles 0-2 in chunk A, 3-4 in chunk B

_COMPILED = None


def _build_program():
    from concourse import bacc, mybir, tile

    f32 = mybir.dt.float32
    f32r = mybir.dt.float32r
    bf16 = mybir.dt.bfloat16
    nc = bacc.Bacc("TRN2", target_bir_lowering=False, debug=False,
                   num_devices=NCORES)

    xka_d = nc.dram_tensor("xka", [128, XK_SPLIT * 256], bf16,
                           kind="ExternalInput").ap()
    xkb_d = nc.dram_tensor("xkb", [128, (KSUB - XK_SPLIT) * 256], bf16,
                           kind="ExternalInput").ap()
    bna_d = nc.dram_tensor("bna", [128, BAND_SPLIT * 384], bf16,
                           kind="ExternalInput").ap()
    bnb_d = nc.dram_tensor("bnb", [128, (TPC - BAND_SPLIT) * 384], bf16,
                           kind="ExternalInput").ap()
    wsa_d = nc.dram_tensor("wsa", [128, WST_SPLIT * OUT], f32r,
                           kind="ExternalInput").ap()
    wsb_d = nc.dram_tensor("wsb", [128, (TPC - WST_SPLIT) * OUT], f32r,
                           kind="ExternalInput").ap()
    out_d = nc.dram_tensor("outp", [128, OUT], f32, kind="ExternalOutput").ap()

    def xk_block(xka_sb, xkb_sb, g):
        if g < XK_SPLIT:
            return xka_sb[:, g * 256:(g + 1) * 256]
        g -= XK_SPLIT
        return xkb_sb[:, g * 256:(g + 1) * 256]

    def band_block(bna_sb, bnb_sb, t, j):
        if t < BAND_SPLIT:
            base = (t * 3 + j) * 128
            return bna_sb[:, base:base + 128]
        base = ((t - BAND_SPLIT) * 3 + j) * 128
        return bnb_sb[:, base:base + 128]

    with tile.TileContext(nc) as tc:
        with (
            tc.tile_pool(name="io", bufs=1) as io,
            tc.tile_pool(name="h1", bufs=TPC) as h1pool,
            tc.tile_pool(name="ps", bufs=3, space="PSUM") as ps,
            tc.tile_pool(name="po", bufs=1, space="PSUM") as po,
            tc.tile_pool(name="jk", bufs=1, space="PSUM") as jk,
        ):
            # --- PE warm-up: junk bf16 matmuls on a zeroed tile.  These
            # ramp the PE HAM clock gate to full rate while the input DMAs
            # are in flight.  Their (all-zero) result is added into the
            # final output tile, which keeps them from being dead-code
            # eliminated without changing the result.
            junk_sb = io.tile([128, 512], bf16, tag="junk")
            nc.gpsimd.memset(junk_sb[:], 0.0)
            junk_ps = jk.tile([128, 512], f32, tag="junkps")
            for _ in range(N_WARM):
                nc.tensor.matmul(junk_ps[:], junk_sb[:, :128], junk_sb[:],
                                 start=True, stop=True)
            # PSUM -> SBUF so the final add has only one PSUM operand
            junk_out = io.tile([128, OUT], f32, tag="junkout")
            nc.vector.tensor_copy(junk_out[:], junk_ps[:, :OUT])

            # --- input DMAs, ordered so tiles 0-1 can start early
            xka_sb = io.tile([128, XK_SPLIT * 256], bf16, tag="xka")
            xkb_sb = io.tile([128, (KSUB - XK_SPLIT) * 256], bf16, tag="xkb")
            bna_sb = io.tile([128, BAND_SPLIT * 384], bf16, tag="bna")
            bnb_sb = io.tile([128, (TPC - BAND_SPLIT) * 384], bf16, tag="bnb")
            wsa_sb = io.tile([128, WST_SPLIT * OUT], f32r, tag="wsa")
            wsb_sb = io.tile([128, (TPC - WST_SPLIT) * OUT], f32r, tag="wsb")
            nc.sync.dma_start(xka_sb[:], xka_d[:])
            nc.sync.dma_start(bna_sb[:], bna_d[:])
            nc.sync.dma_start(xkb_sb[:], xkb_d[:])
            nc.sync.dma_start(bnb_sb[:], bnb_d[:])
            nc.sync.dma_start(wsa_sb[:], wsa_d[:])
            nc.sync.dma_start(wsb_sb[:], wsb_d[:])

            def wst_tile(t):
                if t < WST_SPLIT:
                    return wsa_sb[:, t * OUT:(t + 1) * OUT]
                return wsb_sb[:, (t - WST_SPLIT) * OUT:(t - WST_SPLIT + 1) * OUT]

            # --- SpMM stage: h1 tiles via bf16 band matmuls
            h1_sbs = []
            for t in range(TPC):
                hp = ps.tile([128, 256], f32, tag="h1ps")
                for j in range(3):
                    nc.tensor.matmul(
                        hp[:],
                        band_block(bna_sb, bnb_sb, t, j),
                        xk_block(xka_sb, xkb_sb, t + j),
                        start=(j == 0), stop=(j == 2),
                    )
                # one wide PSUM->SBUF copy; the hi+lo sum folds into the
                # projection (two lhsT slices, same PSUM accumulation)
                hs = h1pool.tile([128, 256], f32r, tag="h1sb")
                nc.vector.tensor_copy(hs[:], hp[:])
                h1_sbs.append(hs)

            # --- projection stage: fp32, PSUM-accumulated over tiles
            op = po.tile([128, OUT], f32, tag="ops")
            for t in range(TPC):
                for half in range(2):
                    nc.tensor.matmul(
                        op[:], h1_sbs[t][:, half * 128:(half + 1) * 128],
                        wst_tile(t),
                        start=(t == 0 and half == 0),
                        stop=(t == TPC - 1 and half == 1),
                    )
            out_sb = io.tile([128, OUT], f32, tag="outsb")
            # op + junk(==0): consumes the warm-up result so it survives DCE
            nc.vector.tensor_add(out_sb[:], op[:], junk_out[:])
            nc.scalar.dma_start(out_d[:], out_sb[:])

    nc.compile()
    return nc


def _get_compiled():
    global _COMPILED
    if _COMPILED is None:
        _COMPILED = _build_program()
    return _COMPILED


def _prep_in_maps(xf, rows, cols, vals, W):
    """Host-side reformat: per-core DRAM arrays (pure data movement)."""
    XT = np.zeros((M_PAD + 2 * 128, B), np.float32)
    XT[128:128 + M] = np.ascontiguousarray(xf[:, :M]).T

    # dense band: Apad[m, k + 128] = A1[m, k]  (duplicates summed)
    Apad = np.zeros((M_PAD, M_PAD + 2 * 128), np.float32)
    np.add.at(Apad, (rows, cols + 128), vals)

    WTpad = np.zeros((M_PAD, OUT), np.float32)
    WTpad[:M] = np.ascontiguousarray(W[:, :M]).T

    in_maps = []
    for c in range(NCORES):
        m0c = CHUNK * c
        # xk hi/lo: (j, p, b) -> [p, j, {hi,lo}, b]
        S = XT[m0c:m0c + KSUB * 128].reshape(KSUB, 128, B)
        hi = S.astype(BF16)
        lo = (S - hi.astype(np.float32)).astype(BF16)
        xkhl = (np.stack([hi, lo], axis=1)       # (j, 2, p, b)
                .transpose(2, 0, 1, 3)           # (p, j, 2, b)
                .reshape(128, KSUB * 256))
        blocks = []
        for t in range(TPC):
            m0t = m0c + 128 * t
            for j in range(3):
                blocks.append(
                    Apad[m0t:m0t + 128, m0t + 128 * j:m0t + 128 * (j + 1)].T)
        bands = np.concatenate(blocks, axis=1).astype(BF16)
        wst = (WTpad[m0c:m0c + CHUNK]
               .reshape(TPC, 128, OUT).transpose(1, 0, 2)
               .reshape(128, TPC * OUT))
        in_maps.append({
            "xka": np.ascontiguousarray(xkhl[:, :XK_SPLIT * 256]),
            "xkb": np.ascontiguousarray(xkhl[:, XK_SPLIT * 256:]),
            "bna": np.ascontiguousarray(bands[:, :BAND_SPLIT * 384]),
            "bnb": np.ascontiguousarray(bands[:, BAND_SPLIT * 384:]),
            "wsa": np.ascontiguousarray(wst[:, :WST_SPLIT * OUT]),
            "wsb": np.ascontiguousarray(wst[:, WST_SPLIT * OUT:]),
        })
    return in_maps


def _run_spmd(in_maps, trace=False):
    from concourse.bass_utils import run_bass_kernel_spmd
    nc = _get_compiled()
    return run_bass_kernel_spmd(nc, in_maps, core_ids=list(range(NCORES)),
                                trace=trace)


def _kernel_impl(x, rows, cols, vals, W, b, trace=False):
    x = np.asarray(x, np.float32)
    rows = np.asarray(rows).astype(np.int64)
    cols = np.asarray(cols).astype(np.int64)
    vals = np.asarray(vals, np.float32)
    W = np.asarray(W, np.float32)
    b = np.asarray(b, np.float32)
    xf = x.reshape(x.shape[0], -1)

    if (rows.size and (max(rows.max(), cols.max()) >= M
                       or np.abs(cols - rows).max() > HALF_BAND)):
        # Structural assumption violated (cannot happen for the deterministic
        # builder, but fall back to an exact host computation just in case).
        h1 = np.zeros((xf.shape[1], xf.shape[0]), np.float32)
        np.add.at(h1, rows, vals[:, None] * xf.T[cols])
        return (h1.T @ W.T + b).astype(np.float32), None

    in_maps = _prep_in_maps(xf, rows, cols, vals, W)
    res = _run_spmd(in_maps, trace=trace)
    acc = np.zeros((B, OUT), np.float32)
    for r in res.results:
        acc += r["outp"]
    return (acc + b[None, :]).astype(np.float32), res


def kernel(x, rows, cols, vals, W, b):
    out, _ = _kernel_impl(x, rows, cols, vals, W, b, trace=False)
    return out


def kernel_traced(x, rows, cols, vals, W, b):
    """Like kernel() but also returns BassKernelResults (exec_time_ns etc.)."""
    return _kernel_impl(x, rows, cols, vals, W, b, trace=True)
